# revision 41
# baseline (speedup 1.0000x reference)
"""Multi-head self-attention (RoPE + causal softmax) Bass kernel for TRN2.

Problem: B=2, H=16, S=2048, D_HEAD=64, fp32 I/O.
Sharding: 32 head-instances (B*H) split 4-per-core across 8 NeuronCores;
no cross-device communication.

v3 design (per core, 4 heads = 2 stacked pairs):
  - Q,K ship host-pre-transposed bf16 with their pair-swapped sign-folded
    shuffles packed alongside (kks/qqs = [128, 2S]); cos|sin packed as one
    [128, 2S] table tensor; V (+ones column) as one 4-head tensor.  All
    loads via the two HWDGE rings (sync + scalar) as a few big DMAs.
  - RoPE on DVE: rot = k*cosT + kshuf*sinT, all-bf16 2x mode, emitted per
    1024-col half into separate half tiles so score matmuls start as soon
    as the first halves are rotated.
  - Scores per 128-row k-tile into [128, 2x512] PSUM (head A | head B),
    causally trimmed; head pair shares the PE via row-group concurrency.
  - exp(s/8): ScalarE exact exp for diagonal tiles + most others; a
    configurable number of old (large-context) k-tiles per chunk use a
    bf16 Schraudolph bit-trick on DVE (i16 = round(s*A+B) viewed as bf16).
  - attn@[V|1] accumulates outT [65, 2x512] over k-tiles; row 64 is the
    softmax denominator.
  - Normalization stays transposed (no PE transposes): copy PSUM->SBUF,
    GpSimd divide -> reciprocal row, PE ones-matmul broadcasts it across
    partitions, DVE multiply -> bf16 [64, q] output, DMA'd transposed;
    host un-transposes.  The PE broadcast is deferred one chunk so the
    in-order PE queue never waits on the divide.
  - Continuous PE warmup bridges the load/RoPE phase.
"""

import math

import numpy as np
import ml_dtypes

import concourse.bass as bass
import concourse.tile as tile
from concourse import bacc, mybir
from concourse.bass_utils import run_bass_kernel_spmd

F32 = mybir.dt.float32
BF16 = mybir.dt.bfloat16
I16 = mybir.dt.int16
EXP = mybir.ActivationFunctionType.Exp
MULT = mybir.AluOpType.mult
ADD = mybir.AluOpType.add
DIV = mybir.AluOpType.divide

B, H, S_FULL, DH = 2, 16, 2048, 64
N_CORES = 8
HEADS_PER_CORE = (B * H) // N_CORES  # 4

# Schraudolph fast-exp constants for bf16 (computing exp(s/8)):
# i16 = round(s * FE_A + FE_B); bits(i16) viewed as bf16 ~= exp(s/8).
FE_A = 128.0 / math.log(2.0) * 0.125
FE_B = 127.0 * 128.0 - 7.38 - 2.0 * 128.0 / math.log(2.0)  # incl -ESH
# exp shift: compute exp(s/8 - ESH) everywhere so fp8e4m3 never saturates
# (numerator and denominator share the factor, the softmax is invariant)
ESH = 2.0
# fp8e4m3 Schraudolph constants for exp(s/8 - ESH) -> int8 bits
F8_A = 8.0 / math.log(2.0) * 0.125
F8_B = 7.0 * 8.0 - 0.5 - ESH * 8.0 / math.log(2.0)
FP8 = mybir.dt.float8e4
I8 = mybir.dt.int8
DR = mybir.MatmulPerfMode.DoubleRow

# Number of oldest non-diagonal k-tiles per chunk whose exp runs on DVE
# via the bit-trick (large-context rows only; diagonal tiles and chunk 0
# always use exact ScalarE exp).
EXP_DVE = 2
# per-(pair, chunk) overrides: later/tail chunks give DVE a bigger share
EXP_DVE_MAP = {}
WARMUP_MMS = 6


# ---------------------------------------------------------------- device IR


def build_nc(n_heads=HEADS_PER_CORE, S=S_FULL, chunk=512, num_devices=N_CORES,
             exp_dve=None, warmup=None):
    NT = S // 128
    npairs = n_heads // 2

    nc = bacc.Bacc(
        "TRN2", target_bir_lowering=False, debug=False, num_devices=num_devices
    )

    qqs = nc.dram_tensor("qqs", [npairs, 128, 2 * S], BF16, kind="ExternalInput").ap()
    kks = nc.dram_tensor("kks", [npairs, 128, 2 * S], BF16, kind="ExternalInput").ap()
    vx = nc.dram_tensor("vx", [n_heads, 128, NT * 65], BF16, kind="ExternalInput").ap()
    vdr = nc.dram_tensor("vdr", [n_heads, 128, (NT // 2) * 2 * 80], FP8,
                         kind="ExternalInput").ap()
    tt = nc.dram_tensor("tt", [128, 2 * S], BF16, kind="ExternalInput").ap()
    tri = nc.dram_tensor("tri", [128, 256], BF16, kind="ExternalInput").ap()
    ident = nc.dram_tensor("ident", [65, 65], F32, kind="ExternalInput").ap()
    o = nc.dram_tensor("o", [n_heads, 128, (S // 128) * DH], BF16,
                       kind="ExternalOutput").ap()

    with tile.TileContext(nc) as tc:
        _body(nc, tc, qqs, kks, vx, vdr, tt, tri, ident, o,
              n_heads=n_heads, S=S, chunk=chunk,
              exp_dve=EXP_DVE if exp_dve is None else exp_dve,
              warmup=WARMUP_MMS if warmup is None else warmup)

    nc.compile()
    return nc


def _body(nc, tc, qqs, kks, vx, vdr, tt, tri, ident, o, *, n_heads, S,
          chunk, exp_dve, warmup):
    from contextlib import ExitStack

    assert chunk == 512
    NT = S // 128
    npairs = n_heads // 2
    nchunks = S // chunk
    kpc = chunk // 128
    HSZ = S // 2
    QS = S // 4

    with ExitStack() as ctx:
        cpool = ctx.enter_context(tc.tile_pool(name="const", bufs=1))
        raw = ctx.enter_context(tc.tile_pool(name="raw", bufs=1))
        rot = ctx.enter_context(tc.tile_pool(name="rot", bufs=1))
        prep = ctx.enter_context(tc.tile_pool(name="prep", bufs=2))
        expp = ctx.enter_context(tc.tile_pool(name="expp", bufs=5))
        sop = ctx.enter_context(tc.tile_pool(name="sop", bufs=2))
        rcp = ctx.enter_context(tc.tile_pool(name="rcp", bufs=4))
        obuf = ctx.enter_context(tc.tile_pool(name="obuf", bufs=1))
        ps_s = ctx.enter_context(tc.tile_pool(name="ps_s", bufs=2, space="PSUM"))
        ps_o = ctx.enter_context(tc.tile_pool(name="ps_o", bufs=1, space="PSUM"))
        ps_t = ctx.enter_context(tc.tile_pool(name="ps_t", bufs=2, space="PSUM"))

        # ---- warmup seed + ACT exp-table preload
        wt = cpool.tile([128, 512], BF16, tag="wt")
        nc.vector.memset(wt[:], 0.25)
        id_t = cpool.tile([65, 65], F32, tag="id")
        biast = cpool.tile([128, 1], F32, tag="biast")
        nc.vector.memset(biast[:], -ESH)
        dme = cpool.tile([128, 8], BF16, tag="dme")
        nc.scalar.activation(dme[:], wt[:, 0:8], EXP, scale=0.125,
                             bias=biast[:])

        # ---- inputs.  Pair-0 + tables stream on the sync HWDGE ring as
        # half-sized 2-segment DMAs into per-half tiles (precise deps);
        # V + pair-1 go on the scalar HWDGE ring.
        # Half tile layout: [128, 2*HSZ] = [nat-half | shuf-half] for kk/qq,
        # [cos-half | sin-half] for tables.
        def q_tiles(pool, tag):
            return [pool.tile([128, 2 * QS], BF16, tag=f"{tag}{g}",
                              name=f"{tag}{g}") for g in range(4)]

        tt_q = q_tiles(cpool, "tt")
        kk0_q = q_tiles(raw, "kk0")
        qq0_q = q_tiles(raw, "qq0")
        tri_t = cpool.tile([128, 256], BF16, tag="tri")
        vall = cpool.tile([128, n_heads * NT * 65], BF16, tag="vall")
        vdrt = cpool.tile([128, n_heads * NT * 80], FP8, tag="vdrt")
        kq1 = []
        for p in range(1, npairs):
            kt_ = raw.tile([128, 2 * S], BF16, tag=f"kk{p}", name=f"kk{p}")
            qt_ = raw.tile([128, 2 * S], BF16, tag=f"qq{p}", name=f"qq{p}")
            kq1.append((kt_, qt_))

        def dma_quarter(eng, t_, dram, g):
            # dst tile [128, 2*QS]; src = dram cols [g*QS:(g+1)*QS] and
            # [S + g*QS : S + (g+1)*QS]
            t3 = t_[:].rearrange("p (g c) -> p g c", c=QS)
            d3 = dram.rearrange("p (g c) -> p g c", c=QS)
            eng.dma_start(t3[:], d3[:, g::4, :])

        # sync ring: quarters 0-1 + small constants; scalar ring: V, then
        # quarters 2-3, then pair-1 bulk + remaining fp8 V.
        for g in (0, 1):
            dma_quarter(nc.sync, tt_q[g], tt, g)
            dma_quarter(nc.sync, kk0_q[g], kks[0], g)
            dma_quarter(nc.sync, qq0_q[g], qqs[0], g)
        nc.sync.dma_start(tri_t[:], tri[:])
        nc.sync.dma_start(id_t[:], ident[:])
        for hh_ in range(n_heads):
            nc.scalar.dma_start(
                vall[:, hh_ * NT * 65:(hh_ + 1) * NT * 65], vx[hh_]
            )
        for g in (2, 3):
            dma_quarter(nc.scalar, tt_q[g], tt, g)
            dma_quarter(nc.scalar, kk0_q[g], kks[0], g)
            dma_quarter(nc.scalar, qq0_q[g], qqs[0], g)
        for hh_ in (0, 1):
            nc.scalar.dma_start(
                vdrt[:, hh_ * NT * 80:(hh_ + 1) * NT * 80], vdr[hh_]
            )
        for p in range(1, npairs):
            nc.scalar.dma_start(kq1[p - 1][0][:], kks[p])
            nc.scalar.dma_start(kq1[p - 1][1][:], qqs[p])
        for hh_ in range(2, n_heads):
            nc.scalar.dma_start(
                vdrt[:, hh_ * NT * 80:(hh_ + 1) * NT * 80], vdr[hh_]
            )

        # ---- PE warmup bridge
        s_d = ps_s.tile([128, 1024], F32, tag="s")
        for _ in range(warmup):
            nc.tensor.matmul(s_d[:, 0:512], wt[0:64, 0:128], wt[0:64, 0:512],
                             start=True, stop=True)

        # ---- RoPE quarter chains, emitted lazily: rot = nat*cos + shuf*sin
        kT = [[rot.tile([128, QS], BF16, tag=f"kT{p}{g}", name=f"kT{p}{g}")
               for g in range(4)] for p in range(npairs)]
        qT = [[rot.tile([128, QS], BF16, tag=f"qT{p}{g}", name=f"qT{p}{g}")
               for g in range(4)] for p in range(npairs)]

        def emit_rope(pr, g):
            if pr == 0:
                srcs = ((kk0_q[g], 0, QS, kT[0][g]),
                        (qq0_q[g], 0, QS, qT[0][g]))
            else:
                kk_, qq_ = kq1[pr - 1]
                srcs = ((kk_, g * QS, S, kT[pr][g]),
                        (qq_, g * QS, S, qT[pr][g]))
            cs = tt_q[g]
            for src_t, lo, sh, dst in srcs:
                t1 = prep.tile([128, QS], BF16, tag="t1")
                t2 = prep.tile([128, QS], BF16, tag="t2")
                nc.vector.tensor_mul(t1[:], src_t[:, lo:lo + QS],
                                     cs[:, 0:QS])
                nc.vector.tensor_mul(t2[:], src_t[:, sh + lo:sh + lo + QS],
                                     cs[:, QS:2 * QS])
                nc.vector.tensor_add(dst[:], t1[:], t2[:])

        emit_rope(0, 0)
        emit_rope(0, 1)
        # rope emission points before chunk (pr, qc): quarters needed soon
        rope_sched = {(0, 1): [(0, 2)], (0, 2): [(0, 3)],
                      (0, 3): [(1, 0), (1, 1)], (1, 1): [(1, 2)],
                      (1, 2): [(1, 3)]}

        def rslice(segs, base, lo, hi):
            g = lo // QS
            assert hi <= (g + 1) * QS, (lo, hi)
            return segs[g][base:base + 64, lo - g * QS:hi - g * QS]

        obs = [obuf.tile([128, NT * DH], BF16, tag=f"ob{h}", name=f"ob{h}")
               for h in range(n_heads)]

        # ---- main loop
        pending_norm = []

        def flush_norm():
            while pending_norm:
                pending_norm.pop(0)()

        stage = []  # cross-chunk deferred mm2 / epilogue closures
        ex8_cur = [None]
        for pr in range(npairs):
            hA, hB = 2 * pr, 2 * pr + 1
            v3A = vall[:, hA * NT * 65:(hA + 1) * NT * 65].rearrange(
                "p (t j) -> p t j", j=65)
            v3B = vall[:, hB * NT * 65:(hB + 1) * NT * 65].rearrange(
                "p (t j) -> p t j", j=65)
            vdA = vdrt[:, hA * NT * 80:(hA + 1) * NT * 80].rearrange(
                "p (t r j) -> p t r j", r=2, j=80)
            vdB = vdrt[:, hB * NT * 80:(hB + 1) * NT * 80].rearrange(
                "p (t r j) -> p t r j", r=2, j=80)
            for qc in range(nchunks):
                for pg in rope_sched.get((pr, qc), []):
                    emit_rope(*pg)
                q0 = qc * chunk
                ktmax = (qc + 1) * kpc
                ndiag = qc * kpc
                exp_dve_c = min(EXP_DVE_MAP.get((pr, qc), exp_dve), ndiag)
                out_t = ps_o.tile([65, 1024], F32, tag="out")
                for kt2 in range(ktmax):
                    rel = max(128 * kt2, q0) - q0
                    s_t = ps_s.tile([128, 1024], F32, tag="s")
                    nc.tensor.matmul(
                        s_t[:, rel:512],
                        rslice(kT[pr], 0, kt2 * 128, (kt2 + 1) * 128),
                        rslice(qT[pr], 0, q0 + rel, q0 + 512),
                        start=True, stop=True,
                    )
                    nc.tensor.matmul(
                        s_t[:, 512 + rel:1024],
                        rslice(kT[pr], 64, kt2 * 128, (kt2 + 1) * 128),
                        rslice(qT[pr], 64, q0 + rel, q0 + 512),
                        start=True, stop=True,
                    )

                    def consume(kt2=kt2, rel=rel, s_t=s_t, ktmax=ktmax, qc=qc,
                                v3A=v3A, v3B=v3B, vdA=vdA, vdB=vdB,
                                out_t=out_t, ndiag=ndiag,
                                exp_dve=exp_dve_c):
                        diag = kt2 >= ndiag
                        last = kt2 == ktmax - 1
                        s3v = s_t[:].rearrange("p (x q) -> p x q", x=2)
                        if not diag and kt2 < exp_dve:
                            # DVE bf16 bit-trick + normal bf16 mm2
                            ex = expp.tile([128, 1024], BF16, tag="ex")

                            def emit_exp():
                                nc.vector.tensor_scalar(
                                    ex[:].bitcast(I16), s_t[:],
                                    FE_A, FE_B, MULT, ADD,
                                )

                            def emit_mm2():
                                for hf, v3 in ((0, v3A), (1, v3B)):
                                    nc.tensor.matmul(
                                        out_t[:, 512 * hf:512 * hf + 512],
                                        v3[:, kt2, :],
                                        ex[:, 512 * hf:512 * hf + 512],
                                        start=(kt2 == 0), stop=False,
                                    )
                            return emit_exp, emit_mm2
                        if not diag:
                            # ACT fp8 plane path; DoubleRow mm2 per kt-pair
                            r = (kt2 - exp_dve) % 2
                            if r == 0:
                                ex8_cur[0] = expp.tile([128, 2048], FP8,
                                                       tag="ex8", name="ex8")
                            ex8 = ex8_cur[0]

                            def emit_exp(ex8=ex8, r=r):
                                e84 = ex8[:].rearrange(
                                    "p (x r n) -> p x r n", x=2, r=2)
                                nc.scalar.activation(
                                    e84[:, :, r, :], s3v, EXP,
                                    scale=0.125, bias=biast[:]
                                )

                            def emit_mm2(ex8=ex8, r=r):
                                if r != 1:
                                    return
                                t = kt2 // 2
                                ex83 = ex8[:].rearrange(
                                    "p (x q) -> p x q", x=2)
                                for hf, vd3 in ((0, vdA), (1, vdB)):
                                    rhs = ex83[:, hf, :].rearrange(
                                        "p (r n) -> p r n", r=2)
                                    nc.tensor.matmul(
                                        out_t[:, 512 * hf:512 * hf + 512],
                                        vd3[:, t, :, 0:65], rhs,
                                        perf_mode=DR,
                                        start=(kt2 == exp_dve + 1
                                               and exp_dve == 0), stop=False,
                                    )
                            return emit_exp, emit_mm2
                        ex = expp.tile([128, 1024], BF16, tag="ex")
                        e3 = ex[:].rearrange("p (x q) -> p x q", x=2)

                        def emit_exp():
                            nc.scalar.activation(
                                e3[:, :, rel:], s3v[:, :, rel:], EXP,
                                scale=0.125, bias=biast[:]
                            )
                            nc.vector.tensor_mul(
                                e3[:, :, rel:rel + 128],
                                e3[:, :, rel:rel + 128],
                                tri_t[:].rearrange("p (x q) -> p x q", x=2),
                            )

                        def emit_mm2():
                            for hf, v3 in ((0, v3A), (1, v3B)):
                                nc.tensor.matmul(
                                    out_t[:, 512 * hf + rel:512 * hf + 512],
                                    v3[:, kt2, :],
                                    ex[:, 512 * hf + rel:512 * hf + 512],
                                    start=(kt2 == 0 and ndiag == 0), stop=last,
                                )
                        return emit_exp, emit_mm2

                    emit_exp, emit_mm2 = consume()
                    emit_exp()
                    stage.append(emit_mm2)
                    if len(stage) > 3:
                        stage.pop(0)()

                def epilogue(qc=qc, out_t=out_t, hA=hA, hB=hB, pr=pr):
                    # drain accumulators promptly (per head half)
                    sos = []
                    for hf in (0, 1):
                        so = sop.tile([65, 512], F32, tag="so",
                                      name=f"so{hf}")
                        nc.vector.tensor_copy(
                            so[:], out_t[:, hf * 512:(hf + 1) * 512])
                        sos.append(so)
                    normA = mknorm(0, hA, sos[0], qc)
                    normB = mknorm(1, hB, sos[1], qc)
                    if pr == npairs - 1 and qc == nchunks - 1:
                        flush_norm()
                        for s_ in normA + normB:
                            s_()
                    else:
                        flush_norm()
                        pending_norm.extend(normA)
                        pending_norm.extend(normB)

                stage.append(epilogue)

                def mknorm(hf, hh, so, qc=qc):
                    # list of small steps so norm work interleaves with the
                    # next chunk's kt rounds (avoids PE-queue convoys)
                    box = {}

                    def step1():
                        box["tr4"] = tr4 = ps_t.tile([128, 4 * 66], F32,
                                                     tag="tr", name="tr4")
                        for j in (0, 1):
                            nc.tensor.transpose(
                                tr4[:, j * 66:j * 66 + 65],
                                so[:, j * 128:(j + 1) * 128], id_t[:],
                            )

                    def step2():
                        tr4 = box["tr4"]
                        for j in (2, 3):
                            nc.tensor.transpose(
                                tr4[:, j * 66:j * 66 + 65],
                                so[:, j * 128:(j + 1) * 128], id_t[:],
                            )

                    def step3():
                        import dataclasses
                        tr4 = box["tr4"]
                        ob = obs[hh]
                        rc = rcp.tile([128, 4], F32, tag="rc")
                        t3 = tr4[:].rearrange("p (j c) -> p j c", c=66)
                        nc.vector.reciprocal(rc[:], t3[:, :, 64])
                        # broadcast reciprocals 64-wide (stride-0 source),
                        # one multiply for the whole chunk
                        rcb = rcp.tile([128, kpc * DH], F32, tag="rcb",
                                       name="rcb")
                        rcv = rc[:].rearrange("p (j o) -> p j o", o=1)
                        rcs = dataclasses.replace(
                            rcv, ap=rcv.ap[:-1] + [[0, DH]])
                        rbv = rcb[:].rearrange("p (j o) -> p j o", o=DH)
                        nc.vector.tensor_copy(rbv[:], rcs)
                        c0 = qc * kpc * DH
                        obv = ob[:, c0:c0 + kpc * DH].rearrange(
                            "p (j o) -> p j o", o=DH)
                        nc.vector.tensor_mul(obv[:], t3[:, :, 0:DH], rbv[:])
                        nc.sync.dma_start(
                            o[hh][:, c0:c0 + kpc * DH], ob[:, c0:c0 + kpc * DH]
                        )
                    return [step1, step2, step3]

        while stage:
            stage.pop(0)()
        flush_norm()


# ---------------------------------------------------------------- host side


def _rope_tables_T(S):
    """Transposed tables cos|sin [128, 2S] for the stacked pair layout."""
    d = np.arange(DH, dtype=np.float32)
    div = np.float32(10000.0) ** ((d // 2 * 2).astype(np.float32) / np.float32(DH))
    pos = np.arange(S, dtype=np.float32)
    ang = pos[None, :] / div[:, None]          # (64, S)
    cosT = np.concatenate([np.cos(ang)] * 2, axis=0)  # (128, S)
    sinT = np.concatenate([np.sin(ang)] * 2, axis=0)
    return np.concatenate([cosT, sinT], axis=1).astype(ml_dtypes.bfloat16)


def host_inputs(qh, kh, vh, S):
    """Per-core input prep.  qh/kh/vh: (n_heads, S, DH) fp32."""
    n_heads = qh.shape[0]
    NT = S // 128
    npairs = n_heads // 2

    def pack(x):
        # (n_heads, S, DH) -> transposed (npairs, 128, S), then pack the
        # pair-swapped sign-folded shuffle alongside -> (npairs, 128, 2S)
        a = x.reshape(npairs, 2, S, DH).transpose(0, 1, 3, 2)  # (pr,2,DH,S)
        xT = np.ascontiguousarray(a.reshape(npairs, 128, S))
        sh = np.empty_like(xT)
        sh[:, 0::2] = -xT[:, 1::2]
        sh[:, 1::2] = xT[:, 0::2]
        return np.concatenate([xT, sh], axis=2)

    qq = pack(qh)
    kk = pack(kh)

    vt = vh.reshape(n_heads, NT, 128, DH).transpose(0, 2, 1, 3)  # (h,128,NT,DH)
    vextf = np.concatenate(
        [vt, np.ones((n_heads, 128, NT, 1), np.float32)], axis=3
    )  # (h, 128, NT, 65)
    vext = vextf.astype(ml_dtypes.bfloat16)
    # DoubleRow fp8 pack: [h, 128, NT/2, 2, 80], k-tile 2t+r in plane r
    # (padded from 65 to 80 so the pair-dim AP step is a multiple of 16)
    vdr5 = np.zeros((n_heads, 128, NT // 2, 2, 80), np.float32)
    vdr5[..., 0:65] = vextf.reshape(n_heads, 128, NT // 2, 2, 65)
    vdr = vdr5.astype(mybir.dt.np(mybir.dt.float8e4))

    tri1 = np.triu(np.ones((128, 128), np.float32))
    tri = np.concatenate([tri1, tri1], axis=1).astype(ml_dtypes.bfloat16)

    bf = ml_dtypes.bfloat16
    return {
        "qqs": qq.astype(bf),
        "kks": kk.astype(bf),
        "vx": np.ascontiguousarray(vext.reshape(n_heads, 128, NT * 65)),
        "vdr": np.ascontiguousarray(vdr.reshape(n_heads, 128, NT * 80)),
        "tt": _rope_tables_T(S),
        "tri": tri,
        "ident": np.eye(65, dtype=np.float32),
    }


_NC_CACHE = {}


def _get_nc():
    if "nc" not in _NC_CACHE:
        _NC_CACHE["nc"] = build_nc()
    return _NC_CACHE["nc"]


def kernel(q, k, v):
    q = np.asarray(q)
    k = np.asarray(k)
    v = np.asarray(v)
    nc = _get_nc()

    qh = q.reshape(B * H, S_FULL, DH)
    kh = k.reshape(B * H, S_FULL, DH)
    vh = v.reshape(B * H, S_FULL, DH)

    in_maps = []
    for c in range(N_CORES):
        sl = slice(c * HEADS_PER_CORE, (c + 1) * HEADS_PER_CORE)
        in_maps.append(host_inputs(qh[sl], kh[sl], vh[sl], S_FULL))

    res = run_bass_kernel_spmd(nc, in_maps, list(range(N_CORES)))

    NT = S_FULL // 128
    out = np.empty((B * H, S_FULL, DH), np.float32)
    for c in range(N_CORES):
        oc = np.asarray(res.results[c]["o"]).astype(np.float32)
        oc = oc.reshape(HEADS_PER_CORE, 128, NT, DH).transpose(0, 2, 1, 3)
        out[c * HEADS_PER_CORE:(c + 1) * HEADS_PER_CORE] = oc.reshape(
            HEADS_PER_CORE, S_FULL, DH
        )
    return out.reshape(B, S_FULL, H * DH)
